# revision 10
# baseline (speedup 1.0000x reference)
"""Trainium2 Bass kernel for nn_ASTGPOLS (GAT + GRU + attention + classifier).

v2: restructured GAT inner loop.
 - Projection matmul extended to N=520: cols 0:512 = W_gat with (c,h)
   head-interleaved output, 512:516 = att_src fold, 516:520 = att_dst fold.
   The self-loop chunk (d=0) supplies a_dst for the tile for free.
 - PSUM->SBUF msg copies split between Scalar and Vector engines.
 - Alpha applied as ONE batched tensor_tensor per chunk-pair using a
   head-broadcast AP (f16, 2x DVE mode) instead of 4 tensor_scalar ops.
 - Aggregation over (d, h) on the tensor engine: identity-matmul
   accumulation into PSUM f32 (stride-4 rhs folds the head mean), lagged
   one tile so the PE never stalls on the DVE.
 - Finalize: STT (0.25*y + b_gat) + activation Lrelu.
Sharding: nodes split 2500/core across 8 cores (dst-sharded graph
partition); host stages per-edge source features in matmul-ready layout.
"""
import math
import time
from contextlib import ExitStack

import numpy as np
import ml_dtypes

import concourse.bass as bass
import concourse.bacc as bacc
import concourse.tile as tile
from concourse import bass_utils, mybir

F16 = mybir.dt.float16
BF16 = mybir.dt.bfloat16
F32 = mybir.dt.float32
I16 = mybir.dt.int16

T, N, E, F, H, C, GH, NCLS = 8, 20000, 160000, 256, 4, 128, 128, 2
NCORES, OWN, NTILE, SLOTS = 8, 2500, 20, 2560
DEAD = SLOTS - OWN          # 60 dead slots (lowest positions, degree 0)
YR = 2688                   # y_dram rows = 21*128 (2500 real + junk zone)
JUNK_ROW = YR - 1
NEG = 0.2
NCH = SLOTS // 512          # 5 column chunks of 512 nodes
WN = 520                    # projection width: 512 msg + 4 a_src + 4 a_dst


def _ap(t, dims):
    return bass.AP(t.tensor, t.offset, dims)


# ---------------------------------------------------------------- host prep
def host_prep(graph, fts):
    """Build per-core staged inputs + the shared degree schedule D[t][j]."""
    graph = np.asarray(graph)
    fts = np.asarray(fts, np.float32)
    f16 = np.float16

    deg = np.empty((T, N), np.int64)
    for t in range(T):
        deg[t] = np.bincount(graph[t, 1], minlength=N) + 1  # + self loop

    # slot orders (per t, per core): dead slots first, then nodes by degree
    slots_all = np.empty((T, NCORES, SLOTS), np.int64)
    D = np.zeros((T, NTILE), np.int64)
    for t in range(T):
        for c in range(NCORES):
            order = np.argsort(deg[t, c * OWN:(c + 1) * OWN], kind="stable") + c * OWN
            s = np.full(SLOTS, -1, np.int64)
            s[DEAD:] = order
            slots_all[t, c] = s
            sd = np.where(s >= 0, deg[t, np.clip(s, 0, None)], 0)
            D[t] = np.maximum(D[t], sd.reshape(NTILE, 128).max(1))
    CH = D.sum(1)                      # chunks per timestep
    CHOFF = np.zeros((T, NTILE), np.int64)
    for t in range(T):
        CHOFF[t] = np.r_[0, np.cumsum(D[t])[:-1]]
    CHTOT = int(CH.sum())
    CHMAX = int(CH.max())

    per_core = []
    for c in range(NCORES):
        fe = np.zeros((CHTOT, 2, 128, 128), f16)
        mask = np.zeros((T, 128, CHMAX), f16)
        sidx = np.zeros((T, 128, 160), np.int16)
        chbase = 0
        for t in range(T):
            src, dst = graph[t]
            slots = slots_all[t, c]
            Dmax = int(D[t].max())
            srcmat = np.full((SLOTS, Dmax), -1, np.int64)
            srcmat[:, 0] = slots                       # self loops
            m = (dst >= c * OWN) & (dst < (c + 1) * OWN)
            s_e, d_e = src[m], dst[m]
            slot_of = np.empty(OWN, np.int64)
            slot_of[np.argsort(deg[t, c * OWN:(c + 1) * OWN], kind="stable")] = \
                np.arange(DEAD, SLOTS)
            sl = slot_of[d_e - c * OWN]
            op = np.argsort(sl, kind="stable")
            sl2, se2 = sl[op], s_e[op]
            first = np.r_[0, np.flatnonzero(np.diff(sl2)) + 1]
            cnt = np.diff(np.r_[first, len(sl2)])
            cc = np.arange(len(sl2)) - np.repeat(first, cnt)
            srcmat[sl2, cc + 1] = se2

            fts_pad = np.vstack([np.zeros((1, F), np.float32), fts[t]])
            cn = np.concatenate(
                [srcmat[j * 128:(j + 1) * 128, :D[t, j]].T for j in range(NTILE)])
            fe_t = fts_pad[cn + 1]                    # [CH_t, 128, 256]
            fe[chbase:chbase + CH[t]] = (
                fe_t.transpose(0, 2, 1).reshape(CH[t], 2, 128, 128).astype(f16))
            mk = np.concatenate(
                [(srcmat[j * 128:(j + 1) * 128, :D[t, j]] >= 0).T
                 for j in range(NTILE)])              # [CH_t, 128]
            mask[t, :, :CH[t]] = mk.T.astype(f16)
            tgt = np.where(slots >= 0, slots - c * OWN, JUNK_ROW).astype(np.int16)
            sidx[t] = np.tile(tgt.reshape(160, 16).T, (8, 1))
            chbase += CH[t]
        per_core.append(dict(fe=fe, mask=mask, sidx=sidx))
    return per_core, D, CH, CHOFF, CHTOT, CHMAX


def host_weights(W_gat, att_src, att_dst, b_gat, W_ih, W_hh, b_ih, b_hh,
                 W_att_in, W_att_out, W_cls, b_cls):
    f16 = np.float16
    W_gat = np.asarray(W_gat, np.float32)          # [256, 512] (h-major cols)
    was = np.stack([W_gat[:, h * C:(h + 1) * C] @ np.asarray(att_src, np.float32)[h]
                    for h in range(H)], 1)            # [256, 4]
    wad = np.stack([W_gat[:, h * C:(h + 1) * C] @ np.asarray(att_dst, np.float32)[h]
                    for h in range(H)], 1)
    d = {}
    # (c,h)-interleave: wge[:, c*4+h] = W_gat[:, h*128+c]
    wi = W_gat.reshape(F, H, C).transpose(0, 2, 1).reshape(F, H * C)
    wge = np.concatenate([wi, was, wad], 1)           # [256, 520]
    d["wge"] = wge.reshape(2, 128, WN).astype(f16)
    d["bgat"] = np.tile(np.asarray(b_gat, np.float32)[None, :], (128, 1)
                        ).astype(np.float32)          # [128, 128] f32
    W_ih = np.asarray(W_ih, np.float32)   # [384, 128]
    W_hh = np.asarray(W_hh, np.float32)
    d["wih"] = np.stack([W_ih[g * 128:(g + 1) * 128, :].T for g in range(3)]
                        ).astype(f16)     # [3, 128(c), 128(g)]
    d["whh"] = np.stack([W_hh[g * 128:(g + 1) * 128, :].T for g in range(3)]
                        ).astype(f16)
    b_ih = np.asarray(b_ih, np.float32)
    b_hh = np.asarray(b_hh, np.float32)
    bias = np.stack([b_ih[0:128] + b_hh[0:128], b_ih[128:256] + b_hh[128:256],
                     b_ih[256:384], b_hh[256:384]], 1)   # [128, 4] r,z,in,hn
    d["biasg"] = bias.astype(np.float32)
    d["wai"] = np.asarray(W_att_in, np.float32).T.astype(f16)          # [c,g]
    Wao = np.asarray(W_att_out, np.float32)    # [128, 256]
    d["wao"] = np.stack([Wao[:, 0:128].T, Wao[:, 128:256].T]).astype(f16)
    d["wcls"] = np.asarray(W_cls, np.float32).T.astype(f16)            # [128,2]
    d["bcls"] = np.tile(np.asarray(b_cls, np.float32)[None, :], (128, 1)
                        ).astype(np.float32)
    d["ident"] = np.eye(128, dtype=f16)
    return d


# ---------------------------------------------------------------- bass build
def build(D, CH, CHOFF, CHTOT, CHMAX):
    nc = bacc.Bacc("TRN2", target_bir_lowering=False, debug=False,
                   enable_asserts=False, num_devices=NCORES)
    dt = nc.dram_tensor
    fe = dt("fe", [CHTOT, 2, 128, 128], F16, kind="ExternalInput").ap()
    mask = dt("mask", [T, 128, CHMAX], F16, kind="ExternalInput").ap()
    sidx = dt("sidx", [T, 128, 160], I16, kind="ExternalInput").ap()
    wge = dt("wge", [2, 128, WN], F16, kind="ExternalInput").ap()
    bgat = dt("bgat", [128, 128], F32, kind="ExternalInput").ap()
    wih = dt("wih", [3, 128, 128], F16, kind="ExternalInput").ap()
    whh = dt("whh", [3, 128, 128], F16, kind="ExternalInput").ap()
    biasg = dt("biasg", [128, 4], F32, kind="ExternalInput").ap()
    wai = dt("wai", [128, 128], F16, kind="ExternalInput").ap()
    wao = dt("wao", [2, 128, 128], F16, kind="ExternalInput").ap()
    wcls = dt("wcls", [128, 2], F16, kind="ExternalInput").ap()
    bcls = dt("bcls", [128, 2], F32, kind="ExternalInput").ap()
    ident = dt("ident", [128, 128], F16, kind="ExternalInput").ap()
    ydr = dt("ydr", [T, YR, 128], F16, kind="Internal").ap()
    out = dt("out", [SLOTS, NCLS], F32, kind="ExternalOutput").ap()

    mult, add, sub = (mybir.AluOpType.mult, mybir.AluOpType.add,
                      mybir.AluOpType.subtract)
    AF = mybir.ActivationFunctionType

    with tile.TileContext(nc) as tc, ExitStack() as top:
        cp = top.enter_context(tc.tile_pool(name="const", bufs=1))

        def cload(ap_in, shape, dtype, src_dims=None, tag=None):
            t_ = cp.tile(shape, dtype, tag=tag)
            src = ap_in if src_dims is None else _ap(ap_in, src_dims)
            nc.sync.dma_start(t_[:], src)
            return t_

        # dram [k, 128, m] -> sbuf [128, k, m]
        def kpm(k, m):
            return [[m, 128], [128 * m, k], [1, m]]

        wge_s = cload(wge[:], [128, 2, WN], F16, kpm(2, WN), tag="wge_s")
        bgat_s = cload(bgat[:], [128, 128], F32, tag="bgat_s")
        wih_s = cload(wih[:], [128, 3, 128], F16, kpm(3, 128), tag="wih_s")
        whh_s = cload(whh[:], [128, 3, 128], F16, kpm(3, 128), tag="whh_s")
        biasg_s = cload(biasg[:], [128, 4], F32, tag="biasg_s")
        wai_s = cload(wai[:], [128, 128], F16, tag="wai_s")
        wao_s = cload(wao[:], [128, 2, 128], F16, kpm(2, 128), tag="wao_s")
        wcls_s = cload(wcls[:], [128, 2], F16, tag="wcls_s")
        bcls_s = cload(bcls[:], [128, 2], F32, tag="bcls_s")
        id_s = cload(ident[:], [128, 128], F16, tag="id_s")

        # zero-fill y_dram
        zt = cp.tile([128, 21, 128], F16)
        nc.vector.memset(zt[:], 0.0)
        for t in range(T):
            dst = _ap(ydr[t], [[128, 128], [128 * 128, 21], [1, 128]])
            nc.sync.dma_start(dst, zt[:])

        # ---------------- GAT per timestep ----------------
        with ExitStack() as gat:
            fep = gat.enter_context(tc.tile_pool(name="fe", bufs=2))
            mkp = gat.enter_context(tc.tile_pool(name="mk", bufs=2))
            sxp = gat.enter_context(tc.tile_pool(name="sx", bufs=2))
            pf = gat.enter_context(tc.tile_pool(name="pf", bufs=3, space="PSUM"))
            pyp = gat.enter_context(tc.tile_pool(name="py", bufs=2, space="PSUM"))
            msgp = gat.enter_context(tc.tile_pool(name="msg", bufs=2))
            scp = gat.enter_context(tc.tile_pool(name="sc", bufs=2))
            alp = gat.enter_context(tc.tile_pool(name="al", bufs=2))
            smp = gat.enter_context(tc.tile_pool(name="sm", bufs=2))
            yp = gat.enter_context(tc.tile_pool(name="y", bufs=2))

            for t in range(T):
                mk_s = mkp.tile([128, CHMAX], F16)
                nc.sync.dma_start(mk_s[:], mask[t])
                sx_s = sxp.tile([128, 160], I16)
                nc.sync.dma_start(sx_s[:], sidx[t])
                y_s = yp.tile([128, NTILE, 128], F16, tag="ys")

                prev = None  # (j, Dj, scaled, y_ps) pending id-accum flush

                def flush(prev):
                    j, Dj, scaled, y_ps = prev
                    for d in range(Dj):
                        nc.tensor.matmul(y_ps[:], id_s[:], scaled[:, d, :],
                                         start=(d == 0), stop=(d == Dj - 1))
                    # head-sum, then y_s[:, j] = lrelu(0.25*y + b_gat)
                    hred = smp.tile([128, 128], F32, tag="hred")
                    yv = _ap(y_ps[:], [y_ps[:].ap[0], [4, 128], [1, 4]])
                    nc.vector.tensor_reduce(hred[:], yv,
                                            axis=mybir.AxisListType.X, op=add)
                    t_f = smp.tile([128, 128], F32, tag="tf")
                    nc.vector.scalar_tensor_tensor(
                        out=t_f[:], in0=hred[:], scalar=0.25, in1=bgat_s[:],
                        op0=mult, op1=add)
                    nc.vector.scalar_tensor_tensor(
                        out=y_s[:, j, :], in0=t_f[:], scalar=NEG, in1=t_f[:],
                        op0=mult, op1=mybir.AluOpType.max)

                for j in range(NTILE):
                    Dj = int(D[t][j])
                    co = int(CHOFF[t][j])
                    chb = int(CH[:t].sum()) + co
                    fe_s = fep.tile([128, Dj, 2, 128], F16, tag="fe")
                    src_ap = _ap(fe[chb], [[128, 128], [2 * 128 * 128, Dj],
                                           [128 * 128, 2], [1, 128]])
                    nc.sync.dma_start(fe_s[:], src_ap)
                    msg = msgp.tile([128, Dj, WN], F16, tag="msg")

                    # pass 1: projections + copies (copy split ACT/DVE)
                    for d in range(Dj):
                        p_f = pf.tile([128, WN], F32, space="PSUM", tag="pf")
                        for kc in range(2):
                            nc.tensor.matmul(p_f[:, 0:512], fe_s[:, d, kc, :],
                                             wge_s[:, kc, 0:512],
                                             start=(kc == 0), stop=(kc == 1))
                        for kc in range(2):
                            nc.tensor.matmul(p_f[:, 512:WN], fe_s[:, d, kc, :],
                                             wge_s[:, kc, 512:WN],
                                             start=(kc == 0), stop=(kc == 1))
                        if d % 5 == 4:
                            nc.vector.tensor_copy(msg[:, d, :], p_f[:])
                        else:
                            nc.scalar.copy(msg[:, d, :], p_f[:])

                    # alpha path (all f16, node-partition space)
                    asrc = bass.AP(msg[:].tensor, msg[:].offset + 512,
                                   [msg[:].ap[0], [WN, Dj], [1, 4]])
                    adst = bass.AP(msg[:].tensor, msg[:].offset + 516,
                                   [msg[:].ap[0], [0, Dj], [1, 4]])
                    apre = alp.tile([128, Dj, 4], F16, tag="apre")
                    nc.vector.tensor_tensor(out=apre[:], in0=asrc, in1=adst,
                                            op=add)
                    nc.vector.scalar_tensor_tensor(
                        out=apre[:], in0=apre[:], scalar=NEG, in1=apre[:],
                        op0=mult, op1=mybir.AluOpType.max)
                    nc.scalar.activation(apre[:], apre[:], AF.Exp)
                    alpha = alp.tile([128, Dj, 4], F16, tag="alpha")
                    mk_b = mk_s[:, co:co + Dj]
                    mk_b = _ap(mk_b, [mk_b.ap[0], mk_b.ap[1], [0, 4]])
                    nc.vector.tensor_tensor(out=alpha[:], in0=apre[:],
                                            in1=mk_b, op=mult)
                    s_ = smp.tile([128, 4], F32, tag="s")
                    a_dh = _ap(alpha[:], [alpha[:].ap[0], [1, 4], [4, Dj]])
                    nc.vector.tensor_reduce(s_[:], a_dh,
                                            axis=mybir.AxisListType.X, op=add)
                    nc.vector.tensor_scalar_add(s_[:], s_[:], 1e-16)
                    inv = smp.tile([128, 4], F32, tag="inv")
                    nc.vector.reciprocal(inv[:], s_[:])
                    alpha2 = alp.tile([128, Dj, 4], F16, tag="alpha2")
                    inv_b = _ap(inv[:], [inv[:].ap[0], [0, Dj], [1, 4]])
                    nc.vector.tensor_tensor(out=alpha2[:], in0=alpha[:],
                                            in1=inv_b, op=mult)

                    # pass 2: batched alpha-mul (pairs of chunks)
                    scaled = scp.tile([128, Dj, 512], F16, tag="scaled")
                    for d0 in range(0, Dj, 2):
                        D2 = min(2, Dj - d0)
                        o_ap = bass.AP(scaled[:].tensor,
                                       scaled[:].offset + d0 * 512,
                                       [scaled[:].ap[0], [512, D2], [4, 128],
                                        [1, 4]])
                        i0 = bass.AP(msg[:].tensor, msg[:].offset + d0 * WN,
                                     [msg[:].ap[0], [WN, D2], [4, 128],
                                      [1, 4]])
                        i1 = bass.AP(alpha2[:].tensor,
                                     alpha2[:].offset + d0 * 4,
                                     [alpha2[:].ap[0], [4, D2], [0, 128],
                                      [1, 4]])
                        nc.vector.tensor_tensor(out=o_ap, in0=i0, in1=i1,
                                                op=mult)

                    if prev is not None:
                        flush(prev)
                    y_ps = pyp.tile([128, 512], F32, space="PSUM", tag="yps")
                    prev = (j, Dj, scaled, y_ps)

                flush(prev)

                nc.gpsimd.dma_scatter_add(
                    ydr[t], y_s[:, :, :], sx_s[:],
                    num_idxs=2560, num_idxs_reg=2560, elem_size=128)

        # ---------------- GRU + attention + classifier ----------------
        with ExitStack() as tail:
            hp = tail.enter_context(tc.tile_pool(name="h", bufs=1))
            ytp = tail.enter_context(tc.tile_pool(name="yt", bufs=2))
            gp = tail.enter_context(tc.tile_pool(name="g", bufs=3))
            pg = tail.enter_context(tc.tile_pool(name="pg", bufs=1, space="PSUM"))
            ctxp = tail.enter_context(tc.tile_pool(name="ctx", bufs=1))
            ptr = tail.enter_context(tc.tile_pool(name="ptr", bufs=2, space="PSUM"))

            h_T = hp.tile([128, SLOTS], F16)
            nc.vector.memset(h_T[:], 0.0)
            ctx = ctxp.tile([128, T, NTILE, 128], F16)

            for t in range(T):
                y_T = ytp.tile([128, SLOTS], F16, tag="yT")
                nc.sync.dma_start_transpose(y_T[:], ydr[t][0:SLOTS, :])
                for ch in range(NCH):
                    ns = slice(ch * 512, (ch + 1) * 512)
                    p_r = pg.tile([128, 512], F32, space="PSUM", tag="pr")
                    p_z = pg.tile([128, 512], F32, space="PSUM", tag="pz")
                    p_gin = pg.tile([128, 512], F32, space="PSUM", tag="pgin")
                    p_ghn = pg.tile([128, 512], F32, space="PSUM", tag="pghn")
                    nc.tensor.matmul(p_r[:], wih_s[:, 0, :], y_T[:, ns],
                                     start=True, stop=False)
                    nc.tensor.matmul(p_r[:], whh_s[:, 0, :], h_T[:, ns],
                                     start=False, stop=True)
                    nc.tensor.matmul(p_z[:], wih_s[:, 1, :], y_T[:, ns],
                                     start=True, stop=False)
                    nc.tensor.matmul(p_z[:], whh_s[:, 1, :], h_T[:, ns],
                                     start=False, stop=True)
                    nc.tensor.matmul(p_gin[:], wih_s[:, 2, :], y_T[:, ns])
                    nc.tensor.matmul(p_ghn[:], whh_s[:, 2, :], h_T[:, ns])
                    r_ = gp.tile([128, 512], F16, tag="r")
                    z_ = gp.tile([128, 512], F16, tag="z")
                    t1 = gp.tile([128, 512], F16, tag="t1")
                    nc.scalar.activation(r_[:], p_r[:], AF.Sigmoid,
                                         bias=biasg_s[:, 0:1])
                    nc.scalar.activation(z_[:], p_z[:], AF.Sigmoid,
                                         bias=biasg_s[:, 1:2])
                    nc.scalar.activation(t1[:], p_ghn[:], AF.Identity,
                                         bias=biasg_s[:, 3:4])
                    nc.vector.tensor_tensor(out=t1[:], in0=r_[:], in1=t1[:],
                                            op=mult)
                    ginb = gp.tile([128, 512], F16, tag="t2")
                    nc.scalar.activation(ginb[:], p_gin[:], AF.Identity,
                                         bias=biasg_s[:, 2:3])
                    nc.vector.tensor_tensor(out=t1[:], in0=t1[:], in1=ginb[:],
                                            op=add)
                    nn_ = gp.tile([128, 512], F16, tag="nn")
                    nc.scalar.activation(nn_[:], t1[:], AF.Tanh)
                    t4 = gp.tile([128, 512], F16, tag="t4")
                    nc.vector.tensor_tensor(out=t4[:], in0=h_T[:, ns], in1=nn_[:],
                                            op=sub)
                    nc.vector.tensor_tensor(out=t4[:], in0=z_[:], in1=t4[:],
                                            op=mult)
                    nc.vector.tensor_tensor(out=h_T[:, ns], in0=nn_[:], in1=t4[:],
                                            op=add)
                # context (node-major) for attention
                for j in range(NTILE):
                    p_t = ptr.tile([128, 128], F16, space="PSUM", tag="ptr")
                    nc.tensor.transpose(p_t[:], h_T[:, j * 128:(j + 1) * 128],
                                        id_s[:])
                    nc.vector.tensor_copy(ctx[:, t, j, :], p_t[:])

            # attention
            ap_ = tail.enter_context(tc.tile_pool(name="at", bufs=2))
            q_T = hp.tile([128, SLOTS], F16)
            for ch in range(NCH):
                ns = slice(ch * 512, (ch + 1) * 512)
                p_q = pg.tile([128, 512], F32, space="PSUM", tag="pr")
                nc.tensor.matmul(p_q[:], wai_s[:], h_T[:, ns])
                nc.scalar.copy(q_T[:, ns], p_q[:])
            qn = hp.tile([128, NTILE, 128], F16)
            for j in range(NTILE):
                p_t = ptr.tile([128, 128], F16, space="PSUM", tag="ptr")
                nc.tensor.transpose(p_t[:], q_T[:, j * 128:(j + 1) * 128], id_s[:])
                nc.vector.tensor_copy(qn[:, j, :], p_t[:])
            mixT = hp.tile([128, SLOTS], F16)
            junk = ap_.tile([128, 128], F16, tag="junk")
            for j in range(NTILE):
                sc = ap_.tile([128, T], F32, tag="sc")
                for t in range(T):
                    nc.vector.tensor_tensor(out=junk[:], in0=qn[:, j, :],
                                            in1=ctx[:, t, j, :], op=mult)
                    nc.vector.tensor_reduce(sc[:, t:t + 1], junk[:],
                                            axis=mybir.AxisListType.X, op=add)
                mx = ap_.tile([128, 1], F32, tag="mx")
                nc.vector.tensor_reduce(mx[:], sc[:], axis=mybir.AxisListType.X,
                                        op=mybir.AluOpType.max)
                ex = ap_.tile([128, T], F32, tag="ex")
                nc.vector.tensor_scalar(out=ex[:], in0=sc[:], scalar1=mx[:, 0:1],
                                        scalar2=None, op0=sub)
                nc.scalar.activation(ex[:], ex[:], AF.Exp)
                ssum = ap_.tile([128, 1], F32, tag="ssum")
                nc.vector.tensor_reduce(ssum[:], ex[:], axis=mybir.AxisListType.X,
                                        op=add)
                nc.vector.reciprocal(ssum[:], ssum[:])
                w_ = ap_.tile([128, T], F16, tag="w")
                nc.vector.tensor_scalar(out=w_[:], in0=ex[:], scalar1=ssum[:, 0:1],
                                        scalar2=None, op0=mult)
                mixj = ap_.tile([128, 128], F16, tag="mixj")
                nc.vector.memset(mixj[:], 0.0)
                for t in range(T):
                    nc.vector.scalar_tensor_tensor(
                        out=mixj[:], in0=ctx[:, t, j, :], scalar=w_[:, t:t + 1],
                        in1=mixj[:], op0=mult, op1=add)
                p_t = ptr.tile([128, 128], F16, space="PSUM", tag="ptr")
                nc.tensor.transpose(p_t[:], mixj[:], id_s[:])
                nc.vector.tensor_copy(mixT[:, j * 128:(j + 1) * 128], p_t[:])

            out_T = hp.tile([128, SLOTS], F16)
            for ch in range(NCH):
                ns = slice(ch * 512, (ch + 1) * 512)
                p_o = pg.tile([128, 512], F32, space="PSUM", tag="pz")
                nc.tensor.matmul(p_o[:], wao_s[:, 0, :], mixT[:, ns],
                                 start=True, stop=False)
                nc.tensor.matmul(p_o[:], wao_s[:, 1, :], q_T[:, ns],
                                 start=False, stop=True)
                nc.scalar.activation(out_T[:, ns], p_o[:], AF.Tanh)
                nc.vector.scalar_tensor_tensor(
                    out=out_T[:, ns], in0=out_T[:, ns], scalar=NEG,
                    in1=out_T[:, ns], op0=mult, op1=mybir.AluOpType.max)
            # classifier + log_softmax
            osb = hp.tile([128, NTILE, NCLS], F32)
            for j in range(NTILE):
                p_l = ptr.tile([128, NCLS], F32, space="PSUM", tag="pl")
                nc.tensor.matmul(p_l[:], out_T[:, j * 128:(j + 1) * 128], wcls_s[:])
                lg = ap_.tile([128, NCLS], F32, tag="lg")
                nc.vector.tensor_tensor(out=lg[:], in0=p_l[:], in1=bcls_s[:],
                                        op=add)
                mx = ap_.tile([128, 1], F32, tag="mx2")
                nc.vector.tensor_reduce(mx[:], lg[:], axis=mybir.AxisListType.X,
                                        op=mybir.AluOpType.max)
                xm = ap_.tile([128, NCLS], F32, tag="xm")
                nc.vector.tensor_scalar(out=xm[:], in0=lg[:], scalar1=mx[:, 0:1],
                                        scalar2=None, op0=sub)
                e2 = ap_.tile([128, NCLS], F32, tag="e2")
                nc.scalar.activation(e2[:], xm[:], AF.Exp)
                s2 = ap_.tile([128, 1], F32, tag="s2")
                nc.vector.tensor_reduce(s2[:], e2[:], axis=mybir.AxisListType.X,
                                        op=add)
                nc.scalar.activation(s2[:], s2[:], AF.Ln)
                nc.vector.tensor_scalar(out=osb[:, j, :], in0=xm[:],
                                        scalar1=s2[:, 0:1], scalar2=None, op0=sub)
            dst = _ap(out[:], [[NCLS, 128], [128 * NCLS, NTILE], [1, NCLS]])
            nc.sync.dma_start(dst, osb[:])

    nc.compile()
    return nc


# ---------------------------------------------------------------- entry
def kernel(graph, fts, time_steps, W_gat, att_src, att_dst, b_gat,
           W_ih, W_hh, b_ih, b_hh, W_att_in, W_att_out, W_cls, b_cls,
           _trace=False, _tmpdir=None):
    per_core, D, CH, CHOFF, CHTOT, CHMAX = host_prep(graph, fts)
    wts = host_weights(W_gat, att_src, att_dst, b_gat, W_ih, W_hh, b_ih, b_hh,
                       W_att_in, W_att_out, W_cls, b_cls)
    nc = build(D, CH, CHOFF, CHTOT, CHMAX)
    in_maps = [{**pc, **wts} for pc in per_core]
    res = bass_utils.run_bass_kernel_spmd(
        nc, in_maps, core_ids=list(range(NCORES)), trace=_trace,
        tmpdir=_tmpdir)
    outs = [r["out"][:OWN] for r in res.results]
    full = np.concatenate(outs, 0).astype(np.float32)
    kernel.last_exec_ns = res.exec_time_ns
    return full
